# revision 26
# baseline (speedup 1.0000x reference)
"""Classwise-ECE kernel for Trainium2 (8 NeuronCores, SPMD data-parallel).

Math
----
For each (class c, bin b) the reference computes
    term = |conf_sum/max(cnt,1) - acc_sum/max(cnt,1)| * cnt/N   (0 when cnt==0)
which simplifies to |conf_sum - acc_sum| / N: the count cancels, and when
cnt==0 both sums are 0 so the term is 0 either way.  Hence

    ECE = mean_c sum_b |Dp[c,b] - Da[c,b]| / N
    Dp[c,b] = sum_n p[n,c]      * [bin(p[n,c]) == b]   (conf_sum)
    Da[c,b] = sum_n [labels[n]==c] * [bin(p[n,c]) == b]  (acc_sum)

Key structural facts:
  * max_c p[n,c] = 1/s_n where s_n = sum_c exp(x[n,c] - max_c x[n,c]).
    If 1/s_n <= 1/15 every element of row n lands in bin 0, so the row's
    entire contribution to Dp is its per-class probability mass (bin 0) and
    its label hit lands in Da[labels[n], 0].
  * Rows with max prob > 1/15 ("flagged", iff s_n < 15) are rare for
    anything but extremely peaked rows; they are re-binned exactly on the
    host from the raw logits.  Correctness holds for ANY input; only the
    (tiny) host correction cost is data-dependent.

Device kernel (per core, rows sharded 8 ways):
  per 128-row tile: rowmax of raw logits (DVE) -> e = exp(x) with the ACT
  accumulator producing s = rowsum(e) for free -> inv = 1/s (DVE) ->
  PE matmul  S[c] += inv^T @ e  accumulated in PSUM across all tiles.
  exp is computed UNSHIFTED: logits are O(10) so exp cannot overflow, and
  e/s is mathematically identical to the max-shifted softmax.  (It also
  keeps the ACT instruction's wait fan-in small - the shifted variant hit
  walrus' "too many sync wait commands" codegen limit.)
  Outputs: S [1,1000], s per row, rowmax per row.
  One full pass over HBM -> memory-roofline bound.

Host: S_total = sum over cores, flags from s, bincount(labels), exact
numpy re-binning of flagged rows, final ECE scalar.
"""

import sys

import numpy as np

for _p in ("/opt/trn_rl_repo",):
    if _p not in sys.path:
        sys.path.append(_p)

N = 131072
C = 1000
N_BINS = 15
N_CORES = 8
P = 128
ROWS_PER_CORE = N // N_CORES          # 16384
NTILES = ROWS_PER_CORE // P           # 128
# Rows with max softmax prob possibly above 1/N_BINS are re-binned exactly
# on the host: flag iff emax*N_BINS > s*(1-MARGIN), where emax = rowmax of
# the bf16 exp tile.  The 2% margin absorbs bf16 rounding and fp32
# summation-order differences; over-flagging only costs host recompute.
FLAG_MARGIN = 2e-2

_NC_CACHE = {}


def _build_bass():
    """Build the per-core Bass program (identical on all 8 cores).

    Raw Bass (no Tile): this toolchain's walrus rejects any instruction
    carrying more than ONE sync-wait, and Tile's semaphore assignment
    freely attaches 2-3.  With explicit engine programs every wait is its
    own standalone instruction, which always lowers cleanly.

    Pipeline per 128-row tile t (slot = t mod depth):
      SP   : [WAR wait act] dma x[slot] <- HBM         .inc dma_sem+16
      ACT  : wait dma, pe;  e[slot]=exp(x[slot]) in BF16,
             accum fp32 s_stage[:,t]                   .inc act
      DVE  : wait act; m_stage[:,t]=rowmax(e[slot]) (bf16 in, f32 out)
             [wait pe] inv[slot]=1/s_stage[:,t] -> bf16  .inc dve x2
      PE   : wait act, dve; psum += inv^T @ e (2 bf16 matmuls, 500 cols)
                                                       .inc pe
    Epilogue: DVE copies psum->S_sb, SP DMAs S_sb/s_stage/m_stage out.

    BF16 e/inv halves PE work (fp32 matmuls lower to hi/lo HW matmul
    pairs) and doubles DVE reduce throughput; PSUM accumulation stays
    fp32.  The resulting perturbation of S is ~1e-3 absolute against
    per-class deviations |S-count| ~ 9, i.e. ~1e-5 relative on the ECE.
    m_stage holds rowmax(e) = exp(rowmax(x)), monotone, used only for
    host-side flagging with a 2% margin that absorbs bf16 rounding.
    """
    from contextlib import ExitStack

    import concourse.bass as bass
    from concourse import mybir

    nc = bass.Bass("TRN2", target_bir_lowering=False, debug=False,
                   num_devices=N_CORES)
    f32 = mybir.dt.float32
    bf16 = mybir.dt.bfloat16
    BUFX, BUFE, BUFI = 12, 6, 4

    x_dram = nc.dram_tensor("logits", [ROWS_PER_CORE, C], f32,
                            kind="ExternalInput").ap()
    S_dram = nc.dram_tensor("S_out", [1, C], f32, kind="ExternalOutput").ap()
    s_dram = nc.dram_tensor("s_out", [P, NTILES], f32,
                            kind="ExternalOutput").ap()
    m_dram = nc.dram_tensor("emax_out", [P, NTILES], bf16,
                            kind="ExternalOutput").ap()

    with ExitStack() as ctx:
        xs = [ctx.enter_context(nc.sbuf_tensor(f"x{i}", [P, C], f32))
              for i in range(BUFX)]
        es = [ctx.enter_context(nc.sbuf_tensor(f"e{i}", [P, C], bf16))
              for i in range(BUFE)]
        invs = [ctx.enter_context(nc.sbuf_tensor(f"inv{i}", [P, 1], bf16))
                for i in range(BUFI)]
        s_stage = ctx.enter_context(
            nc.sbuf_tensor("s_stage", [P, NTILES], f32))
        # bf16 end-to-end: DVE tensor_reduce only hits the 2x packed mode
        # when every src/dst dtype is 2-byte.
        m_stage = ctx.enter_context(
            nc.sbuf_tensor("m_stage", [P, NTILES], bf16))
        S_sb = ctx.enter_context(nc.sbuf_tensor("S_sb", [1, C], f32))
        psum_a = ctx.enter_context(
            nc.psum_tensor("psum_a", [1, 512], f32))
        psum_b = ctx.enter_context(
            nc.psum_tensor("psum_b", [1, 512], f32))
        dma_sem = ctx.enter_context(nc.semaphore(name="dma_sem"))
        act_sem = ctx.enter_context(nc.semaphore(name="act_sem"))
        dve_sem = ctx.enter_context(nc.semaphore(name="dve_sem"))
        pe_sem = ctx.enter_context(nc.semaphore(name="pe_sem"))
        fin_sem = ctx.enter_context(nc.semaphore(name="fin_sem"))
        block = ctx.enter_context(nc.Block())

        @block.sync
        def _(sync):
            for t in range(NTILES):
                if t >= BUFX:
                    # x slot reuse: ACT (exp) is x's only reader.
                    sync.wait_ge(act_sem, t - BUFX + 1)
                sync.dma_start(
                    xs[t % BUFX][:, :], x_dram[t * P:(t + 1) * P, :]
                ).then_inc(dma_sem, 16)
            sync.wait_ge(fin_sem, 1)
            sync.dma_start(S_dram[:, :], S_sb[:, :]).then_inc(dma_sem, 16)
            sync.dma_start(s_dram[:, :], s_stage[:, :]).then_inc(dma_sem, 16)
            sync.dma_start(m_dram[:, :], m_stage[:, :]).then_inc(dma_sem, 16)
            sync.wait_ge(dma_sem, 16 * (NTILES + 3))

        @block.scalar
        def _(scalar):
            for t in range(NTILES):
                scalar.wait_ge(dma_sem, 16 * (t + 1))
                if t >= BUFE:
                    # e slot reuse: PE matmul is the last reader, and its
                    # pe_sem inc transitively covers DVE's rowmax read
                    # (PE waited on the reciprocal, which follows rowmax).
                    scalar.wait_ge(pe_sem, t - BUFE + 1)
                nc.scalar.activation(
                    out=es[t % BUFE][:, :], in_=xs[t % BUFX][:, :],
                    func=mybir.ActivationFunctionType.Exp,
                    accum_out=s_stage[:, t:t + 1],
                ).then_inc(act_sem, 1)

        @block.vector
        def _(vector):
            for t in range(NTILES):
                # One wait covers both DVE reads: e tile and s_stage column
                # are produced by the same ACT instruction.
                vector.wait_ge(act_sem, t + 1)
                nc.vector.tensor_reduce(
                    out=m_stage[:, t:t + 1], in_=es[t % BUFE][:, :],
                    axis=mybir.AxisListType.X, op=mybir.AluOpType.max,
                ).then_inc(dve_sem, 1)
                if t >= BUFI:
                    vector.wait_ge(pe_sem, t - BUFI + 1)  # inv slot reuse
                with nc.allow_low_precision(
                        reason="bf16 1/s weight; ~1e-5 rel impact on ECE"):
                    nc.vector.reciprocal(
                        out=invs[t % BUFI][:, :], in_=s_stage[:, t:t + 1]
                    ).then_inc(dve_sem, 1)
            vector.wait_ge(pe_sem, NTILES)
            nc.vector.tensor_copy(out=S_sb[0:1, 0:500],
                                  in_=psum_a[0:1, 0:500])
            nc.vector.tensor_copy(out=S_sb[0:1, 500:1000],
                                  in_=psum_b[0:1, 0:500]).then_inc(fin_sem, 1)

        @block.tensor
        def _(tensor):
            for t in range(NTILES):
                tensor.wait_ge(act_sem, t + 1)
                tensor.wait_ge(dve_sem, 2 * t + 2)
                first, last = t == 0, t == NTILES - 1
                nc.tensor.matmul(psum_a[0:1, 0:500], invs[t % BUFI][:, :],
                                 es[t % BUFE][:, 0:500],
                                 start=first, stop=last)
                nc.tensor.matmul(psum_b[0:1, 0:500], invs[t % BUFI][:, :],
                                 es[t % BUFE][:, 500:1000],
                                 start=first, stop=last).then_inc(pe_sem, 1)

    return nc


def _get_nc():
    if "nc" not in _NC_CACHE:
        _NC_CACHE["nc"] = _build_bass()
    return _NC_CACHE["nc"]


def _run_device(logits_f32, trace=False):
    """Run the SPMD kernel on 8 cores. Returns (S [1000] f64, s [N] f64,
    emax [N] f64, BassKernelResults)."""
    from concourse.bass_utils import run_bass_kernel_spmd

    nc = _get_nc()
    in_maps = [
        {"logits": np.ascontiguousarray(
            logits_f32[i * ROWS_PER_CORE:(i + 1) * ROWS_PER_CORE])}
        for i in range(N_CORES)
    ]
    res = run_bass_kernel_spmd(nc, in_maps, core_ids=list(range(N_CORES)),
                               trace=trace)
    S = np.zeros(C, np.float64)
    s_parts, m_parts = [], []
    for r in res.results:
        S += r["S_out"][0].astype(np.float64)
        # stage[p, t] holds the value for shard row t*128 + p.
        s_parts.append(r["s_out"].T.reshape(-1).astype(np.float64))
        m_parts.append(r["emax_out"].T.reshape(-1).astype(np.float64))
    return S, np.concatenate(s_parts), np.concatenate(m_parts), res


def _finish_on_host(logits, labels, S, s_rows, emax_rows):
    """Exact ECE from device partials + host re-binning of flagged rows."""
    labels = np.asarray(labels).astype(np.int64)

    Dp = np.zeros((C, N_BINS), np.float64)
    Da = np.zeros((C, N_BINS), np.float64)
    Dp[:, 0] = S
    Da[:, 0] = np.bincount(labels, minlength=C).astype(np.float64)

    flagged = np.nonzero(
        emax_rows * N_BINS > s_rows * (1.0 - FLAG_MARGIN))[0]
    if flagged.size:
        x = np.asarray(logits[flagged], np.float64)
        x -= x.max(axis=1, keepdims=True)
        p = np.exp(x)
        p /= p.sum(axis=1, keepdims=True)
        bins = np.clip(np.ceil(p.astype(np.float32) * N_BINS)
                       .astype(np.int64) - 1, 0, N_BINS - 1)
        # Move these rows' probability mass from bin 0 to their true bins.
        cls = np.broadcast_to(np.arange(C), p.shape)
        Dp[:, 0] -= p.sum(axis=0)
        np.add.at(Dp, (cls.ravel(), bins.ravel()), p.ravel())
        # Move their label hits likewise.
        lab = labels[flagged]
        lab_bins = bins[np.arange(flagged.size), lab]
        np.subtract.at(Da[:, 0], lab, 1.0)
        np.add.at(Da, (lab, lab_bins), 1.0)

    per_class = np.abs(Dp - Da).sum(axis=1) / N
    return np.float32(per_class.mean())


def kernel(logits, labels):
    logits = np.asarray(logits)
    if logits.dtype != np.float32:
        logits = logits.astype(np.float32)
    S, s_rows, emax_rows, _ = _run_device(logits)
    val = _finish_on_host(logits, labels, S, s_rows, emax_rows)
    return np.array(val, dtype=np.float32)
